# revision 52
# baseline (speedup 1.0000x reference)
"""Multi-head attention forward on 8 Trainium2 NeuronCores.

Reference computation (B=2, N=2048, C=1024, H=16, Dh=64):
    qkv = x @ qkv_w.T + qkv_b            -> q, k, v per head
    attn = softmax(q @ k.T / sqrt(Dh))
    out  = (attn @ v) reassembled, then out @ proj_w.T + proj_b

Sharding: 8 cores = 2 batches x 4 head groups (tensor parallel on heads,
data parallel on batch).  Each core computes q/k/v for its 4 heads over
its batch's 2048 tokens, attention for those heads, and a partial
projection using its head-group's rows of proj_w.  The host sums the 4
partial projections per batch (row-parallel TP gather) and adds the
combined bias  proj_b + proj_w @ v_bias  once (the v-bias commutes
through softmax-normalization and the projection, so it never needs to
be applied on-chip).

Design (296us baseline -> ~238us):
  * all matmul operands bf16 (1 row/cycle PE rate, half DMA + SBUF)
  * zero bias matmuls: q/k bias folded into the PSUM->SBUF copy as a
    per-partition tensor_scalar_add; v/proj bias folded into the host
    gather; softmax-denominator ones-columns of v-hat written by one
    memset
  * fine-grained fill: qkv/v-hat production and the projection are
    emitted as single-matmul pieces woven between attention tiles via a
    keyed heap.  The Tile framework's dataflow IS emission order, so a
    piece carries the (group, jc) key of its first consumer and MUST be
    emitted before it; pacing may pull it earlier to plug PE bubbles.
  * each group's softmax normalization is split: the reciprocal chain
    (DVE) is emitted with the group; the PE broadcast + multiply are
    delayed into the next group's pipeline so the PE stream never
    blocks on the reciprocal latency.  Projection pieces enter the fill
    heap from inside that delayed tail (aoT only exists after it).
  * output y stored bf16 per 128-token block, summed f32 on host
  * startup: x/weight DMAs split across the sync+scalar rings, sized so
    the first qk matmuls start as soon as the first x half-chunk lands;
    warmup matmuls tick the PE pstate ramp during the DMA lead-in

On-chip everything runs in the "S^T orientation": S^T[j, i] = sum_d
k^T[d, j] q^T[d, i]; softmax reductions over keys happen via the
ones-column of v-hat (row 64 of the attention-output PSUM tile is the
denominator).  exp runs on ScalarE straight out of PSUM with the
1/sqrt(Dh) scale folded in; max-subtraction is skipped (S ~ N(0,1)).
"""

import sys

if "/opt/trn_rl_repo" not in sys.path:
    sys.path.insert(0, "/opt/trn_rl_repo")

import heapq
from contextlib import ExitStack

import numpy as np
import ml_dtypes

from concourse import bacc, mybir, tile
from concourse.bass_utils import run_bass_kernel_spmd

F32 = mybir.dt.float32
F32R = mybir.dt.float32r
BF16 = mybir.dt.bfloat16
AF = mybir.ActivationFunctionType

B, N, C, H, DH = 2, 2048, 1024, 16, 64
NCORES = 8
HG = 4              # head groups (4 heads each)
HPG = H // HG       # 4 heads per core
DG = HPG * DH       # 256 projected dims per core
CT = C // 128       # 8 contraction tiles
JT = N // 128       # 16 key tiles
IC = N // 512       # 4 query chunks
SCALE = DH ** -0.5

_CACHE = {}
LAST_RESULTS = None


def _build():
    nc = bacc.Bacc("TRN2", target_bir_lowering=False, debug=False,
                   num_devices=NCORES)

    def din(name, shape, dtype=BF16):
        return nc.dram_tensor(name, shape, dtype, kind="ExternalInput").ap()

    xT = din("xT", [C, N])              # batch's x transposed
    wqT = din("wqT", [C, DG])           # this head group's q weights
    wkT = din("wkT", [C, DG])
    wvT = din("wvT", [C, DG])
    wpT = din("wpT", [DG, C])           # head group's rows of proj_w.T
    qb = din("qb", [128, 2], F32)       # q bias, [partition, pair] layout
    kb = din("kb", [128, 2], F32)
    y = nc.dram_tensor("y", [N, C], BF16, kind="ExternalOutput").ap()

    with tile.TileContext(nc) as tc, ExitStack() as ctx:
        # ---- persistent tiles -------------------------------------------
        per = ctx.enter_context(tc.tile_pool(name="per", bufs=1))
        qT_s = per.tile([128, 2, N], BF16, tag="qT")
        kT_s = per.tile([128, 2, N], BF16, tag="kT")
        vh_s = per.tile([128, JT, HPG, DH + 1], BF16, tag="vh")
        aoT_s = per.tile([128, 2, N], BF16, tag="aoT")
        ones_b = per.tile([1, 64], BF16, tag="ones_b")
        nc.vector.memset(ones_b[:], 1.0)
        o64_s = per.tile([1, 16], F32, tag="o64")
        nc.vector.memset(o64_s[:], 1.0)
        qb_s = per.tile([128, 2], F32, tag="qb")
        kb_s = per.tile([128, 2], F32, tag="kb")
        # softmax-denominator ones columns of v-hat, written once
        nc.vector.memset(vh_s[:, :, :, DH:DH + 1], 1.0)
        # warm the exp activation table while the first DMAs run
        warm = per.tile([1, 16], F32, tag="warm")
        nc.scalar.activation(warm[:], o64_s[:], AF.Exp)

        wqT_r = wqT.rearrange("(c p) d -> p c d", p=128)
        wkT_r = wkT.rearrange("(c p) d -> p c d", p=128)
        wvT_r = wvT.rearrange("(c p) d -> p c d", p=128)

        with tc.tile_pool(name="xt", bufs=1) as xt, \
             tc.tile_pool(name="wq1", bufs=2) as wq1, \
             tc.tile_pool(name="wp1", bufs=1) as wp1, \
             tc.tile_pool(name="es2", bufs=6) as es2, \
             tc.tile_pool(name="sm2", bufs=4) as sm2, \
             tc.tile_pool(name="psA", bufs=2, space="PSUM") as psA, \
             tc.tile_pool(name="psS", bufs=2, space="PSUM") as psS, \
             tc.tile_pool(name="psB", bufs=2, space="PSUM") as psB:
            xT_s = xt.tile([128, CT, N], BF16, tag="xT")
            xT_r = xT.rearrange("(c p) n -> p c n", p=128)

            # ---- input DMAs: first q weights + first x chunk lead -------
            # input DMAs split across both hwdge rings; the first x
            # half-chunks are small so the first qk matmuls start early
            wq0_t = wq1.tile([128, CT, 128], BF16, tag="wq")
            nc.sync.dma_start(wq0_t[:], wqT_r[:, :, 0:128])
            nc.sync.dma_start(xT_s[:, 0:4, 0:512], xT_r[:, 0:4, 0:512])
            nc.sync.dma_start(qb_s[:], qb)
            nc.sync.dma_start(kb_s[:], kb)
            nc.sync.dma_start(xT_s[:, 4:8, 0:512], xT_r[:, 4:8, 0:512])
            nc.sync.dma_start(xT_s[:, :, 1024:1536], xT_r[:, :, 1024:1536])
            wp_t = wp1.tile([128, DG // 128, C], BF16, tag="wp")
            nc.sync.dma_start(wp_t[:], wpT.rearrange("(d p) e -> p d e", p=128))
            wv_t = wq1.tile([128, CT, DG], BF16, tag="wv")
            nc.scalar.dma_start(wv_t[:], wvT_r[:])
            wk0_t = wq1.tile([128, CT, 128], BF16, tag="wk")
            nc.scalar.dma_start(wk0_t[:], wkT_r[:, :, 0:128])
            nc.scalar.dma_start(xT_s[:, 0:4, 512:1024], xT_r[:, 0:4, 512:1024])
            nc.scalar.dma_start(xT_s[:, 4:8, 512:1024], xT_r[:, 4:8, 512:1024])
            wq1_t = wq1.tile([128, CT, 128], BF16, tag="wq")
            nc.scalar.dma_start(wq1_t[:], wqT_r[:, :, 128:256])
            wk1_t = wq1.tile([128, CT, 128], BF16, tag="wk")
            nc.scalar.dma_start(wk1_t[:], wkT_r[:, :, 128:256])
            nc.scalar.dma_start(xT_s[:, :, 1536:2048], xT_r[:, :, 1536:2048])

            # ---- production pieces (single-matmul granularity) ----------
            uid = iter(range(10 ** 6))

            def qk_pieces(w_t, b_s, dst, dt, nck):
                box = {}

                def mk(ct):
                    def f():
                        if ct == 0:
                            box["ps"] = psA.tile(
                                [128, 512], F32, tag="mm",
                                name=f"qk{next(uid)}")
                        nc.tensor.matmul(
                            box["ps"][:], w_t[:, ct, :],
                            xT_s[:, ct, nck * 512:(nck + 1) * 512],
                            start=(ct == 0), stop=(ct == CT - 1))
                        if ct == CT - 1:
                            nc.vector.tensor_scalar_add(
                                dst[:, dt, nck * 512:(nck + 1) * 512],
                                box["ps"][:], b_s[:, dt:dt + 1])
                    return f
                return [mk(ct) for ct in range(CT)]

            def vh_pieces(jt):
                box = {}

                def mk(ct):
                    def f():
                        if ct == 0:
                            box["ps"] = psA.tile(
                                [128, DG], F32, tag="mm",
                                name=f"vh{next(uid)}")
                        nc.tensor.matmul(
                            box["ps"][:], xT_s[:, ct, jt * 128:(jt + 1) * 128],
                            wv_t[:, ct, :],
                            start=(ct == 0), stop=(ct == CT - 1))
                        if ct == CT - 1:
                            for h in range(HPG):
                                nc.vector.tensor_copy(
                                    vh_s[:, jt, h, 0:DH],
                                    box["ps"][:, h * DH:(h + 1) * DH])
                    return f
                return [mk(ct) for ct in range(CT)]

            def proj_pieces(it, on_act=False):
                # one 128-token output block: 2 col-halves x 2 accum mms,
                # one batched [128,1024] copy + store.  The tail blocks
                # drain their PSUM via the otherwise-idle ACT engine.
                boxes = {}

                def mk(ec, dt):
                    def f():
                        if dt == 0:
                            boxes[ec] = psA.tile(
                                [128, 512], F32, tag="mm",
                                name=f"pj_{it}_{ec}")
                        nc.tensor.matmul(
                            boxes[ec][:],
                            aoT_s[:, dt, it * 128:(it + 1) * 128],
                            wp_t[:, dt, ec * 512:(ec + 1) * 512],
                            start=(dt == 0), stop=(dt == 1))
                        if dt == 1:
                            if ec == 0:
                                boxes["y"] = y3.tile(
                                    [128, 1024], BF16, tag="y",
                                    name=f"y_{it}")
                            y_t = boxes["y"]
                            if on_act:
                                # tail blocks: drain the two PSUM halves on
                                # ACT and DVE in parallel; one store per
                                # block keeps the sync-ring issue count low
                                cp = nc.scalar.copy if ec == 0 \
                                    else nc.vector.tensor_copy
                                cp(y_t[:, ec * 512:(ec + 1) * 512],
                                   boxes[ec][:])
                                if ec == 1:
                                    nc.sync.dma_start(
                                        y[it * 128:(it + 1) * 128, :],
                                        y_t[:])
                            else:
                                nc.vector.tensor_copy(
                                    y_t[:, ec * 512:(ec + 1) * 512],
                                    boxes[ec][:])
                                if ec == 1:
                                    nc.sync.dma_start(
                                        y[it * 128:(it + 1) * 128, :],
                                        y_t[:])
                    return f
                return [mk(ec, dt) for ec in range(2) for dt in range(2)]

            # ---- attention ----------------------------------------------
            # fills: deque of ((group, jc), piece, early_ok) sorted by key;
            # a piece MUST be emitted before the attention tile its key
            # names (emission order IS the dataflow), and MAY be emitted
            # earlier for PE pacing when early_ok.
            seq = iter(range(10 ** 9))

            def fpush(fills, key, f, early):
                heapq.heappush(fills, (key, next(seq), f, early))

            def drain_until(fills, key):
                while fills and fills[0][0] <= key:
                    heapq.heappop(fills)[2]()

            def pace(fills, key):
                if fills:
                    k, _, f, early = fills[0]
                    if early or k <= key:
                        heapq.heappop(fills)
                        f()

            def attention(p, ic, gi, fills, npull, tail_prev=None,
                          pace_cap=None, last=False):
                i0 = ic * 512
                pulls = [0]
                outs = [psB.tile([65, 512], F32, tag="outT",
                                 name=f"o{p}_{ic}{s}") for s in "ab"]
                for jc in range(JT):
                    if jc == 1 and tail_prev is not None:
                        tail_prev()
                        tail_prev = None
                    drain_until(fills, (gi, jc))
                    for _ in range(npull + (3 if jc == 0 else 0)):
                        if pace_cap is None or pulls[0] < pace_cap:
                            if fills:
                                pulls[0] += 1
                            pace(fills, (gi, jc))
                    st = psS.tile([128, 1024], F32, tag="st")
                    nc.tensor.matmul(st[:, 0:512],
                                     kT_s[0:64, p, jc * 128:(jc + 1) * 128],
                                     qT_s[0:64, p, i0:i0 + 512],
                                     start=True, stop=True)
                    nc.tensor.matmul(st[:, 512:1024],
                                     kT_s[64:128, p, jc * 128:(jc + 1) * 128],
                                     qT_s[64:128, p, i0:i0 + 512],
                                     start=True, stop=True)
                    es = es2.tile([128, 1024], BF16, tag="es")
                    nc.scalar.activation(es[:], st[:], AF.Exp, scale=SCALE)
                    nc.tensor.matmul(outs[0][:],
                                     vh_s[:, jc, 2 * p, :], es[:, 0:512],
                                     start=(jc == 0), stop=(jc == JT - 1))
                    nc.tensor.matmul(outs[1][:],
                                     vh_s[:, jc, 2 * p + 1, :],
                                     es[:, 512:1024],
                                     start=(jc == 0), stop=(jc == JT - 1))
                for _ in range(2):
                    pace(fills, (gi, JT))
                # g0's DVE queue is deep in production copies and g7's
                # reciprocal chain is the tail's critical path; in both,
                # ACT is idle right after the last exp, so the copies of
                # the chain run there and only the reciprocal stays DVE
                fast = gi in (0, 7)
                recs = []
                for hi in range(2):
                    den = sm2.tile([1, 512], F32, tag="den")
                    if fast:
                        nc.scalar.copy(den[:], outs[hi][64:65, :])
                    else:
                        nc.vector.tensor_copy(den[:], outs[hi][64:65, :])
                    rec = sm2.tile([1, 512], F32, tag="rec")
                    nc.vector.reciprocal_approx_fast(rec[:], den[:])
                    rec_b = sm2.tile([1, 512], BF16, tag="rec_b")
                    if fast:
                        nc.scalar.copy(rec_b[:], rec[:])
                    else:
                        nc.vector.tensor_copy(rec_b[:], rec[:])
                    recs.append(rec_b)

                def tail():
                    bc = psA.tile([128, 512], F32, tag="mm")
                    for hi in range(2):
                        nc.tensor.matmul(bc[hi * 64:hi * 64 + 64, :],
                                         ones_b[:], recs[hi][:],
                                         start=True, stop=True)
                        ao = aoT_s[hi * 64:hi * 64 + 64, p, i0:i0 + 512]
                        if last and hi == 0:
                            nc.scalar.copy(ao, outs[hi][0:64, :])
                        else:
                            nc.vector.tensor_copy(ao, outs[hi][0:64, :])
                    aof = aoT_s[:, p, i0:i0 + 512]
                    nc.vector.tensor_mul(aof, aof, bc[:])
                    # projection over this chunk becomes available only
                    # once the normalized aoT exists
                    if p == 1:
                        for it in range(4 * ic, 4 * (ic + 1)):
                            for f in proj_pieces(it, on_act=(it >= 12)):
                                fpush(fills, (99, 0), f, True)
                return tail

            # ---- emission schedule --------------------------------------
            # Pre-attention head: enough production for attention(0,0) to
            # start (q/k pair-0 chunk 0, v-hat tiles 0-3).  Everything else
            # becomes fill pieces pulled between attention tiles.
            with tc.tile_pool(name="y3", bufs=2) as y3:
                wps = psA.tile([128, 512], F32, tag="mm", name="warmps")
                for i in range(36):
                    nc.tensor.matmul(wps[0:64, 0:64], ones_b[:], ones_b[:],
                                     start=(i == 0), stop=(i == 35))
                for f in qk_pieces(wq0_t, qb_s, qT_s, 0, 0):
                    f()
                for i in range(16):
                    nc.tensor.matmul(wps[0:64, 0:64], ones_b[:], ones_b[:],
                                     start=(i == 0), stop=(i == 15))
                for f in qk_pieces(wk0_t, kb_s, kT_s, 0, 0):
                    f()
                for f in vh_pieces(0):
                    f()

                fills = []
                # key (gi, jc) = first attention tile consuming the piece
                # under group order (0,0)(0,1)(1,0)(1,1)(0,2)(1,2)(0,3)(1,3)
                for jt in range(1, 16):
                    for f in vh_pieces(jt):
                        fpush(fills, (0, jt), f, True)
                for nck in range(1, IC):
                    for f in qk_pieces(wk0_t, kb_s, kT_s, 0, nck):
                        fpush(fills, (0, 4 * nck), f, True)
                for nck in range(1, IC):
                    for f in qk_pieces(wq0_t, qb_s, qT_s, 0, nck):
                        fpush(fills, (nck, 0), f, True)
                for f in qk_pieces(wk1_t, kb_s, kT_s, 1, 0):
                    fpush(fills, (4, 0), f, True)
                for f in qk_pieces(wq1_t, qb_s, qT_s, 1, 0):
                    fpush(fills, (4, 0), f, True)
                for nck in range(1, IC):
                    for f in qk_pieces(wk1_t, kb_s, kT_s, 1, nck):
                        fpush(fills, (4, 4 * nck), f, True)
                for nck in range(1, IC):
                    for f in qk_pieces(wq1_t, qb_s, qT_s, 1, nck):
                        fpush(fills, (4 + nck, 0), f, True)

                order = [(0, 0), (0, 1), (0, 2), (0, 3),
                         (1, 0), (1, 1), (1, 2), (1, 3)]
                npulls = [0, 1, 1, 1, 2, 4, 4, 4]
                tail = None
                for gi, (p, ic) in enumerate(order):
                    tail = attention(p, ic, gi, fills, npulls[gi], tail,
                                     pace_cap=(10 if gi == 7 else None),
                                     last=(gi == 7))
                while fills:
                    heapq.heappop(fills)[2]()
                tail()
                while fills:
                    heapq.heappop(fills)[2]()

    nc.compile()
    return nc


def _get_nc():
    if "nc" not in _CACHE:
        _CACHE["nc"] = _build()
    return _CACHE["nc"]


def kernel(x, qkv_w, qkv_b, proj_w, proj_b):
    global LAST_RESULTS
    x = np.asarray(x, dtype=np.float32)
    qkv_w = np.asarray(qkv_w, dtype=np.float32)
    qkv_b = np.asarray(qkv_b, dtype=np.float32)
    proj_w = np.asarray(proj_w, dtype=np.float32)
    proj_b = np.asarray(proj_b, dtype=np.float32)

    nc = _get_nc()

    bf16 = ml_dtypes.bfloat16
    # host-side sharding / layout prep (transposition + slicing + casts)
    xT = [np.ascontiguousarray(x[b].T).astype(bf16) for b in range(B)]
    wqT_f = qkv_w[0:C].T                # [C, C]
    wkT_f = qkv_w[C:2 * C].T
    wvT_f = qkv_w[2 * C:3 * C].T
    wpT_f = proj_w.T                    # [C, C]
    in_maps = []
    for c in range(NCORES):
        b, g = divmod(c, HG)
        ds, de = g * DG, (g + 1) * DG
        in_maps.append({
            "xT": xT[b],
            "wqT": np.ascontiguousarray(wqT_f[:, ds:de]).astype(bf16),
            "wkT": np.ascontiguousarray(wkT_f[:, ds:de]).astype(bf16),
            "wvT": np.ascontiguousarray(wvT_f[:, ds:de]).astype(bf16),
            "wpT": np.ascontiguousarray(wpT_f[ds:de, :]).astype(bf16),
            "qb": np.ascontiguousarray(
                qkv_b[ds:de].reshape(2, 128).T),
            "kb": np.ascontiguousarray(
                qkv_b[C + ds:C + de].reshape(2, 128).T),
        })

    LAST_RESULTS = run_bass_kernel_spmd(nc, in_maps, list(range(NCORES)))
    # unshard: sum the 4 partial projections per batch (row-parallel TP
    # gather) and apply the combined bias  proj_b + proj_w @ v_bias
    bias = proj_b + proj_w @ qkv_b[2 * C:3 * C]
    out = np.empty((B, N, C), np.float32)
    for b in range(B):
        acc = LAST_RESULTS.results[b * HG]["y"].astype(np.float32)
        for g in range(1, HG):
            acc = acc + LAST_RESULTS.results[b * HG + g]["y"].astype(np.float32)
        out[b] = acc + bias
    return out


# revision 53
# speedup vs baseline: 1.1826x; 1.1826x over previous
"""Multi-head attention forward on 8 Trainium2 NeuronCores.

Reference computation (B=2, N=2048, C=1024, H=16, Dh=64):
    qkv = x @ qkv_w.T + qkv_b            -> q, k, v per head
    attn = softmax(q @ k.T / sqrt(Dh))
    out  = (attn @ v) reassembled, then out @ proj_w.T + proj_b

Sharding: 8 cores = 2 batches x 4 head groups (tensor parallel on heads,
data parallel on batch).  Each core computes q/k/v for its 4 heads over
its batch's 2048 tokens, attention for those heads, and a partial
projection using its head-group's rows of proj_w.  The host sums the 4
partial projections per batch (row-parallel TP gather) and adds the
combined bias  proj_b + proj_w @ v_bias  once (the v-bias commutes
through softmax-normalization and the projection, so it never needs to
be applied on-chip).

Design (296us baseline -> ~238us):
  * all matmul operands bf16 (1 row/cycle PE rate, half DMA + SBUF)
  * zero bias matmuls: q/k bias folded into the PSUM->SBUF copy as a
    per-partition tensor_scalar_add; v/proj bias folded into the host
    gather; softmax-denominator ones-columns of v-hat written by one
    memset
  * fine-grained fill: qkv/v-hat production and the projection are
    emitted as single-matmul pieces woven between attention tiles via a
    keyed heap.  The Tile framework's dataflow IS emission order, so a
    piece carries the (group, jc) key of its first consumer and MUST be
    emitted before it; pacing may pull it earlier to plug PE bubbles.
  * each group's softmax normalization is split: the reciprocal chain
    (DVE) is emitted with the group; the PE broadcast + multiply are
    delayed into the next group's pipeline so the PE stream never
    blocks on the reciprocal latency.  Projection pieces enter the fill
    heap from inside that delayed tail (aoT only exists after it).
  * output y stored bf16 per 128-token block, summed f32 on host
  * startup: x/weight DMAs split across the sync+scalar rings, sized so
    the first qk matmuls start as soon as the first x half-chunk lands;
    warmup matmuls tick the PE pstate ramp during the DMA lead-in

On-chip everything runs in the "S^T orientation": S^T[j, i] = sum_d
k^T[d, j] q^T[d, i]; softmax reductions over keys happen via the
ones-column of v-hat (row 64 of the attention-output PSUM tile is the
denominator).  exp runs on ScalarE straight out of PSUM with the
1/sqrt(Dh) scale folded in; max-subtraction is skipped (S ~ N(0,1)).
"""

import sys

if "/opt/trn_rl_repo" not in sys.path:
    sys.path.insert(0, "/opt/trn_rl_repo")

import heapq
from contextlib import ExitStack

import numpy as np
import ml_dtypes

from concourse import bacc, mybir, tile
from concourse.bass_utils import run_bass_kernel_spmd

F32 = mybir.dt.float32
F32R = mybir.dt.float32r
BF16 = mybir.dt.bfloat16
AF = mybir.ActivationFunctionType

B, N, C, H, DH = 2, 2048, 1024, 16, 64
NCORES = 8
HG = 4              # head groups (4 heads each)
HPG = H // HG       # 4 heads per core
DG = HPG * DH       # 256 projected dims per core
CT = C // 128       # 8 contraction tiles
JT = N // 128       # 16 key tiles
IC = N // 512       # 4 query chunks
SCALE = DH ** -0.5

_CACHE = {}
LAST_RESULTS = None


def _build():
    nc = bacc.Bacc("TRN2", target_bir_lowering=False, debug=False,
                   num_devices=NCORES)

    def din(name, shape, dtype=BF16):
        return nc.dram_tensor(name, shape, dtype, kind="ExternalInput").ap()

    xT = din("xT", [C, N])              # batch's x transposed
    wqT = din("wqT", [C, DG])           # this head group's q weights
    wkT = din("wkT", [C, DG])
    wvT = din("wvT", [C, DG])
    wpT = din("wpT", [DG, C])           # head group's rows of proj_w.T
    qb = din("qb", [128, 2], F32)       # q bias, [partition, pair] layout
    kb = din("kb", [128, 2], F32)
    y = nc.dram_tensor("y", [N, C], BF16, kind="ExternalOutput").ap()

    with tile.TileContext(nc) as tc, ExitStack() as ctx:
        # ---- persistent tiles -------------------------------------------
        per = ctx.enter_context(tc.tile_pool(name="per", bufs=1))
        qT_s = per.tile([128, 2, N], BF16, tag="qT")
        kT_s = per.tile([128, 2, N], BF16, tag="kT")
        vh_s = per.tile([128, JT, HPG, DH + 1], BF16, tag="vh")
        aoT_s = per.tile([128, 2, N], BF16, tag="aoT")
        ones_b = per.tile([1, 64], BF16, tag="ones_b")
        nc.vector.memset(ones_b[:], 1.0)
        o64_s = per.tile([1, 16], F32, tag="o64")
        nc.vector.memset(o64_s[:], 1.0)
        qb_s = per.tile([128, 2], F32, tag="qb")
        kb_s = per.tile([128, 2], F32, tag="kb")
        # softmax-denominator ones columns of v-hat, written once
        nc.vector.memset(vh_s[:, :, :, DH:DH + 1], 1.0)
        # warm the exp activation table while the first DMAs run
        warm = per.tile([1, 16], F32, tag="warm")
        nc.scalar.activation(warm[:], o64_s[:], AF.Exp)

        wqT_r = wqT.rearrange("(c p) d -> p c d", p=128)
        wkT_r = wkT.rearrange("(c p) d -> p c d", p=128)
        wvT_r = wvT.rearrange("(c p) d -> p c d", p=128)

        with tc.tile_pool(name="xt", bufs=1) as xt, \
             tc.tile_pool(name="wq1", bufs=2) as wq1, \
             tc.tile_pool(name="wp1", bufs=1) as wp1, \
             tc.tile_pool(name="es2", bufs=6) as es2, \
             tc.tile_pool(name="sm2", bufs=4) as sm2, \
             tc.tile_pool(name="psA", bufs=2, space="PSUM") as psA, \
             tc.tile_pool(name="psS", bufs=2, space="PSUM") as psS, \
             tc.tile_pool(name="psB", bufs=2, space="PSUM") as psB:
            xT_s = xt.tile([128, CT, N], BF16, tag="xT")
            xT_r = xT.rearrange("(c p) n -> p c n", p=128)

            # ---- input DMAs: first q weights + first x chunk lead -------
            # input DMAs split across both hwdge rings; the first x
            # half-chunks are small so the first qk matmuls start early
            wq0_t = wq1.tile([128, CT, 128], BF16, tag="wq")
            nc.sync.dma_start(wq0_t[:], wqT_r[:, :, 0:128])
            nc.sync.dma_start(xT_s[:, 0:4, 0:512], xT_r[:, 0:4, 0:512])
            nc.sync.dma_start(qb_s[:], qb)
            nc.sync.dma_start(kb_s[:], kb)
            nc.sync.dma_start(xT_s[:, 4:8, 0:512], xT_r[:, 4:8, 0:512])
            nc.sync.dma_start(xT_s[:, :, 1024:1536], xT_r[:, :, 1024:1536])
            wp_t = wp1.tile([128, DG // 128, C], BF16, tag="wp")
            nc.sync.dma_start(wp_t[:], wpT.rearrange("(d p) e -> p d e", p=128))
            wv_t = wq1.tile([128, CT, DG], BF16, tag="wv")
            nc.scalar.dma_start(wv_t[:], wvT_r[:])
            wk0_t = wq1.tile([128, CT, 128], BF16, tag="wk")
            nc.scalar.dma_start(wk0_t[:], wkT_r[:, :, 0:128])
            nc.scalar.dma_start(xT_s[:, 0:4, 512:1024], xT_r[:, 0:4, 512:1024])
            nc.scalar.dma_start(xT_s[:, 4:8, 512:1024], xT_r[:, 4:8, 512:1024])
            wq1_t = wq1.tile([128, CT, 128], BF16, tag="wq")
            nc.scalar.dma_start(wq1_t[:], wqT_r[:, :, 128:256])
            wk1_t = wq1.tile([128, CT, 128], BF16, tag="wk")
            nc.scalar.dma_start(wk1_t[:], wkT_r[:, :, 128:256])
            nc.scalar.dma_start(xT_s[:, :, 1536:2048], xT_r[:, :, 1536:2048])

            # ---- production pieces (single-matmul granularity) ----------
            uid = iter(range(10 ** 6))

            def qk_pieces(w_t, b_s, dst, dt, nck):
                box = {}

                def mk(ct):
                    def f():
                        if ct == 0:
                            box["ps"] = psA.tile(
                                [128, 512], F32, tag="mm",
                                name=f"qk{next(uid)}")
                        nc.tensor.matmul(
                            box["ps"][:], w_t[:, ct, :],
                            xT_s[:, ct, nck * 512:(nck + 1) * 512],
                            start=(ct == 0), stop=(ct == CT - 1))
                        if ct == CT - 1:
                            nc.vector.tensor_scalar_add(
                                dst[:, dt, nck * 512:(nck + 1) * 512],
                                box["ps"][:], b_s[:, dt:dt + 1])
                    return f
                return [mk(ct) for ct in range(CT)]

            def vh_pieces(jt):
                box = {}

                def mk(ct):
                    def f():
                        if ct == 0:
                            box["ps"] = psA.tile(
                                [128, DG], F32, tag="mm",
                                name=f"vh{next(uid)}")
                        nc.tensor.matmul(
                            box["ps"][:], xT_s[:, ct, jt * 128:(jt + 1) * 128],
                            wv_t[:, ct, :],
                            start=(ct == 0), stop=(ct == CT - 1))
                        if ct == CT - 1:
                            for h in range(HPG):
                                nc.vector.tensor_copy(
                                    vh_s[:, jt, h, 0:DH],
                                    box["ps"][:, h * DH:(h + 1) * DH])
                    return f
                return [mk(ct) for ct in range(CT)]

            def proj_pieces(it, on_act=False):
                # one 128-token output block: 2 col-halves x 2 accum mms,
                # one batched [128,1024] copy + store.  The tail blocks
                # drain their PSUM via the otherwise-idle ACT engine.
                boxes = {}

                def mk(ec, dt):
                    def f():
                        if dt == 0:
                            boxes[ec] = psA.tile(
                                [128, 512], F32, tag="mm",
                                name=f"pj_{it}_{ec}")
                        nc.tensor.matmul(
                            boxes[ec][:],
                            aoT_s[:, dt, it * 128:(it + 1) * 128],
                            wp_t[:, dt, ec * 512:(ec + 1) * 512],
                            start=(dt == 0), stop=(dt == 1))
                        if dt == 1:
                            if ec == 0:
                                boxes["y"] = y3.tile(
                                    [128, 1024], BF16, tag="y",
                                    name=f"y_{it}")
                            y_t = boxes["y"]
                            if on_act:
                                # tail blocks: drain the two PSUM halves on
                                # ACT and DVE in parallel; one store per
                                # block keeps the sync-ring issue count low
                                cp = nc.scalar.copy if ec == 0 \
                                    else nc.vector.tensor_copy
                                cp(y_t[:, ec * 512:(ec + 1) * 512],
                                   boxes[ec][:])
                                if ec == 1:
                                    nc.sync.dma_start(
                                        y[it * 128:(it + 1) * 128, :],
                                        y_t[:])
                            else:
                                nc.vector.tensor_copy(
                                    y_t[:, ec * 512:(ec + 1) * 512],
                                    boxes[ec][:])
                                if ec == 1:
                                    nc.sync.dma_start(
                                        y[it * 128:(it + 1) * 128, :],
                                        y_t[:])
                    return f
                return [mk(ec, dt) for ec in range(2) for dt in range(2)]

            # ---- attention ----------------------------------------------
            # fills: deque of ((group, jc), piece, early_ok) sorted by key;
            # a piece MUST be emitted before the attention tile its key
            # names (emission order IS the dataflow), and MAY be emitted
            # earlier for PE pacing when early_ok.
            seq = iter(range(10 ** 9))

            def fpush(fills, key, f, early):
                heapq.heappush(fills, (key, next(seq), f, early))

            def drain_until(fills, key):
                while fills and fills[0][0] <= key:
                    heapq.heappop(fills)[2]()

            def pace(fills, key):
                if fills:
                    k, _, f, early = fills[0]
                    if early or k <= key:
                        heapq.heappop(fills)
                        f()

            def attention(p, ic, gi, fills, npull, tail_prev=None,
                          pace_cap=None, last=False):
                i0 = ic * 512
                pulls = [0]
                outs = [psB.tile([65, 512], F32, tag="outT",
                                 name=f"o{p}_{ic}{s}") for s in "ab"]
                for jc in range(JT):
                    if jc == 1 and tail_prev is not None:
                        tail_prev()
                        tail_prev = None
                    drain_until(fills, (gi, jc))
                    for _ in range(npull + (3 if jc == 0 else 0)):
                        if pace_cap is None or pulls[0] < pace_cap:
                            if fills:
                                pulls[0] += 1
                            pace(fills, (gi, jc))
                    st = psS.tile([128, 1024], F32, tag="st")
                    nc.tensor.matmul(st[:, 0:512],
                                     kT_s[0:64, p, jc * 128:(jc + 1) * 128],
                                     qT_s[0:64, p, i0:i0 + 512],
                                     start=True, stop=True)
                    nc.tensor.matmul(st[:, 512:1024],
                                     kT_s[64:128, p, jc * 128:(jc + 1) * 128],
                                     qT_s[64:128, p, i0:i0 + 512],
                                     start=True, stop=True)
                    es = es2.tile([128, 1024], BF16, tag="es")
                    nc.scalar.activation(es[:], st[:], AF.Exp, scale=SCALE)
                    nc.tensor.matmul(outs[0][:],
                                     vh_s[:, jc, 2 * p, :], es[:, 0:512],
                                     start=(jc == 0), stop=(jc == JT - 1))
                    nc.tensor.matmul(outs[1][:],
                                     vh_s[:, jc, 2 * p + 1, :],
                                     es[:, 512:1024],
                                     start=(jc == 0), stop=(jc == JT - 1))
                for _ in range(2):
                    pace(fills, (gi, JT))
                # g0's DVE queue is deep in production copies and g7's
                # reciprocal chain is the tail's critical path; in both,
                # ACT is idle right after the last exp, so the copies of
                # the chain run there and only the reciprocal stays DVE
                fast = gi in (0, 7)
                recs = []
                for hi in range(2):
                    den = sm2.tile([1, 512], F32, tag="den")
                    if fast:
                        nc.scalar.copy(den[:], outs[hi][64:65, :])
                    else:
                        nc.vector.tensor_copy(den[:], outs[hi][64:65, :])
                    rec = sm2.tile([1, 512], F32, tag="rec")
                    nc.vector.reciprocal_approx_fast(rec[:], den[:])
                    rec_b = sm2.tile([1, 512], BF16, tag="rec_b")
                    if fast:
                        nc.scalar.copy(rec_b[:], rec[:])
                    else:
                        nc.vector.tensor_copy(rec_b[:], rec[:])
                    recs.append(rec_b)

                def tail():
                    bc = psA.tile([128, 512], F32, tag="mm")
                    for hi in range(2):
                        nc.tensor.matmul(bc[hi * 64:hi * 64 + 64, :],
                                         ones_b[:], recs[hi][:],
                                         start=True, stop=True)
                        ao = aoT_s[hi * 64:hi * 64 + 64, p, i0:i0 + 512]
                        if last and hi == 0:
                            nc.scalar.copy(ao, outs[hi][0:64, :])
                        else:
                            nc.vector.tensor_copy(ao, outs[hi][0:64, :])
                    aof = aoT_s[:, p, i0:i0 + 512]
                    nc.vector.tensor_mul(aof, aof, bc[:])
                    # projection over this chunk becomes available only
                    # once the normalized aoT exists
                    if p == 1:
                        for it in range(4 * ic, 4 * (ic + 1)):
                            for f in proj_pieces(it, on_act=(it >= 12)):
                                fpush(fills, (99, 0), f, True)
                return tail

            # ---- emission schedule --------------------------------------
            # Pre-attention head: enough production for attention(0,0) to
            # start (q/k pair-0 chunk 0, v-hat tiles 0-3).  Everything else
            # becomes fill pieces pulled between attention tiles.
            with tc.tile_pool(name="y3", bufs=2) as y3:
                wps = psA.tile([128, 512], F32, tag="mm", name="warmps")
                for i in range(36):
                    nc.tensor.matmul(wps[0:64, 0:64], ones_b[:], ones_b[:],
                                     start=(i == 0), stop=(i == 35))
                for f in qk_pieces(wq0_t, qb_s, qT_s, 0, 0):
                    f()
                for i in range(40):
                    nc.tensor.matmul(wps[0:64, 0:64], ones_b[:], ones_b[:],
                                     start=(i == 0), stop=(i == 39))
                for f in qk_pieces(wk0_t, kb_s, kT_s, 0, 0):
                    f()
                for f in vh_pieces(0):
                    f()

                fills = []
                # key (gi, jc) = first attention tile consuming the piece
                # under group order (0,0)(0,1)(1,0)(1,1)(0,2)(1,2)(0,3)(1,3)
                for jt in range(1, 16):
                    for f in vh_pieces(jt):
                        fpush(fills, (0, jt), f, True)
                for nck in range(1, IC):
                    for f in qk_pieces(wk0_t, kb_s, kT_s, 0, nck):
                        fpush(fills, (0, 4 * nck), f, True)
                for nck in range(1, IC):
                    for f in qk_pieces(wq0_t, qb_s, qT_s, 0, nck):
                        fpush(fills, (nck, 0), f, True)
                for f in qk_pieces(wk1_t, kb_s, kT_s, 1, 0):
                    fpush(fills, (4, 0), f, True)
                for f in qk_pieces(wq1_t, qb_s, qT_s, 1, 0):
                    fpush(fills, (4, 0), f, True)
                for nck in range(1, IC):
                    for f in qk_pieces(wk1_t, kb_s, kT_s, 1, nck):
                        fpush(fills, (4, 4 * nck), f, True)
                for nck in range(1, IC):
                    for f in qk_pieces(wq1_t, qb_s, qT_s, 1, nck):
                        fpush(fills, (4 + nck, 0), f, True)

                order = [(0, 0), (0, 1), (0, 2), (0, 3),
                         (1, 0), (1, 1), (1, 2), (1, 3)]
                npulls = [0, 1, 1, 1, 2, 4, 4, 4]
                tail = None
                for gi, (p, ic) in enumerate(order):
                    tail = attention(p, ic, gi, fills, npulls[gi], tail,
                                     pace_cap=(10 if gi == 7 else None),
                                     last=(gi == 7))
                while fills:
                    heapq.heappop(fills)[2]()
                tail()
                while fills:
                    heapq.heappop(fills)[2]()

    nc.compile()
    return nc


def _get_nc():
    if "nc" not in _CACHE:
        _CACHE["nc"] = _build()
    return _CACHE["nc"]


def kernel(x, qkv_w, qkv_b, proj_w, proj_b):
    global LAST_RESULTS
    x = np.asarray(x, dtype=np.float32)
    qkv_w = np.asarray(qkv_w, dtype=np.float32)
    qkv_b = np.asarray(qkv_b, dtype=np.float32)
    proj_w = np.asarray(proj_w, dtype=np.float32)
    proj_b = np.asarray(proj_b, dtype=np.float32)

    nc = _get_nc()

    bf16 = ml_dtypes.bfloat16
    # host-side sharding / layout prep (transposition + slicing + casts)
    xT = [np.ascontiguousarray(x[b].T).astype(bf16) for b in range(B)]
    wqT_f = qkv_w[0:C].T                # [C, C]
    wkT_f = qkv_w[C:2 * C].T
    wvT_f = qkv_w[2 * C:3 * C].T
    wpT_f = proj_w.T                    # [C, C]
    in_maps = []
    for c in range(NCORES):
        b, g = divmod(c, HG)
        ds, de = g * DG, (g + 1) * DG
        in_maps.append({
            "xT": xT[b],
            "wqT": np.ascontiguousarray(wqT_f[:, ds:de]).astype(bf16),
            "wkT": np.ascontiguousarray(wkT_f[:, ds:de]).astype(bf16),
            "wvT": np.ascontiguousarray(wvT_f[:, ds:de]).astype(bf16),
            "wpT": np.ascontiguousarray(wpT_f[ds:de, :]).astype(bf16),
            "qb": np.ascontiguousarray(
                qkv_b[ds:de].reshape(2, 128).T),
            "kb": np.ascontiguousarray(
                qkv_b[C + ds:C + de].reshape(2, 128).T),
        })

    LAST_RESULTS = run_bass_kernel_spmd(nc, in_maps, list(range(NCORES)))
    # unshard: sum the 4 partial projections per batch (row-parallel TP
    # gather) and apply the combined bias  proj_b + proj_w @ v_bias
    bias = proj_b + proj_w @ qkv_b[2 * C:3 * C]
    out = np.empty((B, N, C), np.float32)
    for b in range(B):
        acc = LAST_RESULTS.results[b * HG]["y"].astype(np.float32)
        for g in range(1, HG):
            acc = acc + LAST_RESULTS.results[b * HG + g]["y"].astype(np.float32)
        out[b] = acc + bias
    return out
